# revision 12
# baseline (speedup 1.0000x reference)
"""Trainium2 Bass kernel for multi-head quadratic spatial attention.

Problem: q,k,v [b=8, heads=8, h=32, w=32, d=64] fp32; full attention over
the 1024-position spatial grid independently per (b, head); output
[b, h, w, heads*d].

Sharding: data-parallel over batch — core c handles b=c (8 heads of
[1024, 64] attention per core), no cross-core communication.

Per-core pipeline (matmuls bf16 with fp32 PSUM accumulation; the
normalization path stays fp32). Heads are processed in PAIRS (A, B) so
the d=64 contraction can be packed into the 128-row PE array:
  - load Q,K,V natural [1024, 64], cast to bf16 during SWDGE DMA
  - PE-transpose Q,K to d-major; head A lands on PSUM rows 0:64, head B
    on rows 64:128 (col-tiled — the two transposes run concurrently),
    giving packed Qt/Kt pair tiles [128, 1024]
  - mm1 row-tiled: head A contracts on array rows 0:64, head B on rows
    64:128 concurrently -> St pair PSUM [128, 2048] fp32 (A|B)
  - one ScalarE exp(St * d^-1/2) per j-block pair -> Pt bf16 [128, 2048]
  - mm2: lhsT = [V | 1] j-chunk [128, 65] bf16, rhs = Pt slices ->
    accumulate PSUM Ot [65, 512] per i-half; row 64 = softmax sums
  - PE-transpose Ot back per i-block (fp32), DVE reciprocal +
    tensor_scalar mult, DMA out natural layout
"""

from contextlib import ExitStack

import numpy as np

F32 = None
BF16 = None
FP16 = None

_cache = {}


def _imports():
    global F32, BF16, FP16
    import concourse.bass as bass
    import concourse.tile as tile
    from concourse import mybir
    from concourse.masks import make_identity

    F32 = mybir.dt.float32
    BF16 = mybir.dt.bfloat16
    FP16 = mybir.dt.float16
    return bass, tile, mybir, make_identity


def _split_multi_waits(nc, mybir):
    """Walrus in this container supports only ONE sync-wait per instruction.
    Hoist extra waits onto same-engine InstNoOp's inserted just before."""
    ctr = 0
    for f in nc.m.functions:
        for bb in f.blocks:
            insts = bb.instructions
            if not any(
                i.sync_info and i.sync_info.on_wait and len(i.sync_info.on_wait) > 1
                for i in insts
            ):
                continue
            out = []
            for inst in insts:
                si = inst.sync_info
                waits = list(si.on_wait) if si and si.on_wait else []
                if len(waits) > 1:
                    for w in waits[:-1]:
                        ctr += 1
                        nop = mybir.InstNoOp(
                            name=f"I-wsplit-{ctr}",
                            engine=inst.engine,
                            ins=[],
                            outs=[],
                            sync_info=mybir.SyncInfo(on_wait=[w], on_update=[]),
                        )
                        nc.register_instruction(nop)
                        out.append(nop)
                    si.on_wait = waits[-1:]
                out.append(inst)
            bb.instructions = out


def _build_nc(heads=8, seq=1024, d=64):
    bass, tile, mybir, make_identity = _imports()
    assert heads % 2 == 0 and seq % 1024 == 0 and d == 64
    nt = seq // 128          # i/j blocks of 128
    nh = seq // 512          # i-halves of 512
    dv = d + 1
    scale = float(d) ** -0.5

    nc = bass.Bass(trn_type="TRN2", target_bir_lowering=False)
    q_d = nc.dram_tensor("q", [heads, seq, d], F32, kind="ExternalInput")
    k_d = nc.dram_tensor("k", [heads, seq, d], F32, kind="ExternalInput")
    v_d = nc.dram_tensor("v", [heads, seq, d], F32, kind="ExternalInput")
    o_d = nc.dram_tensor("out", [seq, heads * d], F32, kind="ExternalOutput")

    q_ap = q_d[:].rearrange("n (t p) d -> n p t d", p=128)
    k_ap = k_d[:].rearrange("n (t p) d -> n p t d", p=128)
    v_ap = v_d[:].rearrange("n (t p) d -> n p t d", p=128)
    o_ap = o_d[:].rearrange("(t p) c -> p t c", p=128)

    with tile.TileContext(nc) as tc, ExitStack() as ctx:
        consts = ctx.enter_context(tc.tile_pool(name="consts", bufs=1))
        nat = ctx.enter_context(tc.tile_pool(name="nat", bufs=2))
        dmaj = ctx.enter_context(tc.tile_pool(name="dmaj", bufs=2))
        ptp = ctx.enter_context(tc.tile_pool(name="ptp", bufs=18))
        otp = ctx.enter_context(tc.tile_pool(name="otp", bufs=3))
        outp = ctx.enter_context(tc.tile_pool(name="outp", bufs=3))
        small = ctx.enter_context(tc.tile_pool(name="small", bufs=4))

        tp_ps = ctx.enter_context(tc.tile_pool(name="tp_ps", bufs=2, space="PSUM"))
        st_ps = ctx.enter_context(tc.tile_pool(name="st_ps", bufs=1, space="PSUM"))
        oa_ps = ctx.enter_context(tc.tile_pool(name="oa_ps", bufs=2, space="PSUM"))

        ident_bf = consts.tile([128, 128], BF16)
        make_identity(nc, ident_bf[:])
        ident_f16 = consts.tile([128, 128], FP16)
        make_identity(nc, ident_f16[:])

        def load_and_transpose(pair):
            """DMA pair inputs (bf16 cast) and build packed d-major tiles:
            head A on partitions 0:64, head B on 64:128 (col-tiled PE
            transposes run concurrently)."""
            st8 = {"heads": (2 * pair, 2 * pair + 1), "v": [], "pts": [],
                   "oacc": {}, "ostage": {}}
            # pair-interleaved natural tiles: [..., 2, d] with head A at
            # index 0 and head B at index 1, so one [128, 128] PE transpose
            # of a block yields A's d-rows on partitions 0:64 and B's on
            # 64:128 — exactly the packed pair layout mm1 wants.
            # Q/K ride the low-latency HWDGE path as fp32 and are cast to
            # bf16 by one DVE copy; V keeps the SWDGE cast-during-DMA path.
            qs = nat.tile([128, nt, 2, d], F32, tag="qs")
            ks = nat.tile([128, nt, 2, d], F32, tag="ks")
            qp = nat.tile([128, nt, 2, d], BF16, tag="qp")
            kp = nat.tile([128, nt, 2, d], BF16, tag="kp")
            for idx, n in enumerate(st8["heads"]):
                nc.sync.dma_start(out=qs[:, :, idx, :], in_=q_ap[n])
            nc.vector.tensor_copy(out=qp[:], in_=qs[:])
            for idx, n in enumerate(st8["heads"]):
                nc.sync.dma_start(out=ks[:, :, idx, :], in_=k_ap[n])
            nc.vector.tensor_copy(out=kp[:], in_=ks[:])
            for idx, n in enumerate(st8["heads"]):
                vn = nat.tile([128, nt, dv], BF16, tag=f"v_nat{idx}")
                # ones column for the softmax-denominator trick
                nc.vector.memset(vn[:], 1.0)
                nc.gpsimd.dma_start(out=vn[:, :, 0:d], in_=v_ap[n])
                st8["v"].append(vn)
            qt = dmaj.tile([128, seq], BF16, tag="qt")
            kt = dmaj.tile([128, seq], BF16, tag="kt")
            for src, dst in ((qp, qt), (kp, kt)):
                for g in range(nt // 4):
                    tp = tp_ps.tile([128, 512], BF16, tag="tp")
                    for u in range(4):
                        t = g * 4 + u
                        nc.tensor.transpose(
                            tp[:, u * 128 : (u + 1) * 128],
                            src[:, t, :, :],
                            ident_bf[:],
                        )
                    nc.vector.tensor_copy(
                        out=dst[:, g * 512 : (g + 1) * 512], in_=tp[:]
                    )
            st8["qt"], st8["kt"] = qt, kt
            return st8

        def mm1_exp(s, jb):
            """Row-tiled pair mm1 into a [128, 2*seq] St tile + one exp."""
            qt, kt = s["qt"], s["kt"]
            st = st_ps.tile([128, 2 * seq], F32, tag="st")
            for c in range(nh):
                nc.tensor.matmul(
                    st[:, c * 512 : (c + 1) * 512],
                    kt[0:64, jb * 128 : (jb + 1) * 128],
                    qt[0:64, c * 512 : (c + 1) * 512],
                    start=True,
                    stop=True,
                )
                nc.tensor.matmul(
                    st[:, seq + c * 512 : seq + (c + 1) * 512],
                    kt[64:128, jb * 128 : (jb + 1) * 128],
                    qt[64:128, c * 512 : (c + 1) * 512],
                    start=True,
                    stop=True,
                )
            pt = ptp.tile([128, 2 * seq], BF16, tag="pt")
            nc.scalar.activation(
                out=pt[:],
                in_=st[:],
                func=mybir.ActivationFunctionType.Exp,
                scale=scale,
            )
            s["pts"].append(pt)

        def mm2_slot(s, slot):
            """One PE-stream slot of the pair's mm2: 4 accumulating
            matmuls of group (head, half) = slot//2; epilogue on the
            closing slot. Only one oacc group is live at a time so PSUM
            stays within budget."""
            g = slot // 2
            phase = slot % 2
            idx, half = g // 2, g % 2
            if phase == 0:
                s["oacc"][g] = oa_ps.tile([dv, 512], F32, name="oacc", tag="oacc")
            oacc = s["oacc"][g]
            off = idx * seq + half * 512
            for jj in range(4):
                jb = phase * 4 + jj
                nc.tensor.matmul(
                    oacc[:],
                    s["v"][idx][:, jb, :],
                    s["pts"][jb][:, off : off + 512],
                    start=(jb == 0),
                    stop=(jb == nt - 1),
                )
            if phase == 1:
                _epilogue(s, idx, half, oacc)

        def _epilogue(s, idx, half, oacc):
            n = s["heads"][idx]
            if idx not in s["ostage"]:
                s["ostage"][idx] = outp.tile([128, nt, d], F32, name="ostage", tag="ostage")
            ostage = s["ostage"][idx]
            ot = otp.tile([dv, 512], FP16, tag="ot")
            nc.vector.tensor_copy(out=ot[:], in_=oacc[:])
            for u in range(4):
                t = half * 4 + u
                ob = tp_ps.tile([128, dv], FP16, tag="tp")
                nc.tensor.transpose(
                    ob[:], ot[:, u * 128 : (u + 1) * 128], ident_f16[0:dv, 0:dv]
                )
                rec = small.tile([128, 1], F32, tag="rec")
                nc.vector.reciprocal(out=rec[:], in_=ob[:, d : d + 1])
                nc.vector.tensor_scalar_mul(ostage[:, t, :], ob[:, 0:d], rec[:])
            if half == nh - 1:
                nc.sync.dma_start(
                    out=o_ap[:, :, n * d : (n + 1) * d], in_=ostage[:]
                )

        # software pipeline: pair p's mm1/exp interleaved with pair p-1's mm2
        prev = None
        for pair in range(heads // 2):
            cur = load_and_transpose(pair)
            for jb in range(nt):
                mm1_exp(cur, jb)
                if prev is not None:
                    mm2_slot(prev, jb)
            prev = cur
        for jb in range(nt):
            mm2_slot(prev, jb)

    _split_multi_waits(nc, mybir)
    return nc


def _get_nc():
    if "nc" not in _cache:
        _cache["nc"] = _build_nc()
    return _cache["nc"]


def _run(q, k, v, trace=False):
    from concourse.bass_utils import run_bass_kernel_spmd

    b, heads, h, w, d = 8, 8, 32, 32, 64
    q = np.ascontiguousarray(np.asarray(q, dtype=np.float32))
    k = np.ascontiguousarray(np.asarray(k, dtype=np.float32))
    v = np.ascontiguousarray(np.asarray(v, dtype=np.float32))
    assert q.shape == (b, heads, h, w, d), q.shape

    nc = _get_nc()
    in_maps = [
        {
            "q": q[c].reshape(heads, h * w, d),
            "k": k[c].reshape(heads, h * w, d),
            "v": v[c].reshape(heads, h * w, d),
        }
        for c in range(b)
    ]
    res = run_bass_kernel_spmd(nc, in_maps, core_ids=list(range(b)), trace=trace)
    out = np.stack(
        [res.results[c]["out"].reshape(h, w, heads * d) for c in range(b)]
    )
    return out, res


def kernel(q, k, v):
    out, _ = _run(q, k, v)
    return out
